# revision 32
# baseline (speedup 1.0000x reference)
"""Grouped GEMM (MoE block-diagonal) on 8 Trainium2 NeuronCores.

Problem: x [262144, 256] bf16, w [1024, 256] bf16 (G=8 experts of [128, 256]).
Rows g*32768:(g+1)*32768 of x belong to expert g.
Output [262144, 1024] bf16, block-diagonal: out[rows_g, g*128:(g+1)*128] = x_g @ w_g^T.

Strategy (expert-parallel):
  - Core g gets expert g: x_g [32768, 256] and w_g [128, 256].
  - Host packs both operands so the contraction dim K lands on SBUF
    partitions (PE matmul contracts over the partition dim) AND every load
    DMA moves ONE contiguous run per partition (per chunk of L tokens at
    token T: xP[p, colbase + h*L + t] = x_g^T[h*128+p, T+t], a 2*L*2-byte
    run).  4 KiB runs cap the HWDGE load stream at ~240 GB/s (~73
    ns/packet fixed cost); the 16 KiB runs of the 4096-token chunks let
    the 16 SDMA engines reach the HBM roofline (~420 GB/s observed).
  - Device computes yT_g [128, 32768] = w_g @ x_g^T with lhsT = w_g^T
    (stationary, both K-halves resident) and rhs = packed x columns,
    512-token matmuls accumulating K over 2 halves into [128,1024] PSUM.
  - Loads and stores taper at the end (final chunks 2048/1024/1024, final
    stores 4096/2048/1024/1024 tokens) so the compute+store tail after the
    load stream ends is short.
  - Host transposes yT_g back and scatters into the zero-filled
    block-diagonal output (the zero blocks never touch the device).
"""

import sys

for _p in ("/opt/trn_rl_repo", "/root/.axon_site/_ro/trn_rl_repo"):
    if _p not in sys.path:
        sys.path.insert(0, _p)

import numpy as np

G = 8          # experts == cores
K = 256        # contraction dim
N = 128        # output dim per expert
M = 262144     # total tokens
MPC = M // G   # tokens per core = 32768

MT = 8192      # tokens per tile
PT = 512       # tokens per matmul (max PE free dim)
PB = 1024      # tokens per PSUM tile

# Load-chunk schedule per tile (token counts, each a PB multiple). The bulk
# uses 4096-token chunks (16 KiB contiguous per partition -> near-line-rate
# descriptors); the final tile tapers so the last chunk's compute+store tail
# after the load stream ends is short.
TILE_CHUNKS = [
    [2048, 2048, 4096],
    [4096, 4096],
    [4096, 4096],
    [4096, 2048, 1024, 1024],
]
# Store boundaries per tile (exclusive token ends within the tile); the last
# tile tapers so the final store is small and lands right after its cast.
TILE_STORES = [
    [8192],
    [8192],
    [8192],
    [2048, 4096, 6144, 7168],
]


def _split_multi_waits(nc, mybir):
    """This walrus build rejects any instruction carrying more than one sync
    wait ("Too many sync wait commands", setupSyncWait). Hoist all but one
    wait of each offender onto fresh single-wait EventSemaphore instructions
    placed just before it on the same engine queue — semantically identical
    (sequencer-level blocking, monotonic sem conditions)."""
    for fn in nc.m.functions:
        for blk in fn.blocks:
            new_insts = []
            for inst in blk.instructions:
                si = getattr(inst, "sync_info", None)
                waits = list(si.on_wait) if si is not None and si.on_wait else []
                if len(waits) > 1:
                    for w in waits[:-1]:
                        name = nc.get_next_instruction_name()
                        ev = mybir.InstEventSemaphore(
                            name=name,
                            engine=inst.engine,
                            ins=[],
                            outs=[],
                            sync_info=mybir.SyncInfo(on_wait=[w], on_update=[]),
                        )
                        nc.inst_map[name] = ev
                        new_insts.append(ev)
                    si.on_wait = waits[-1:]
                new_insts.append(inst)
            blk.instructions = new_insts


def _patch_new_backend():
    import concourse.bass_utils as bu

    if getattr(bu, "_new_backend_patched", False):
        return
    bu._new_backend_patched = True
    orig = bu.get_walrus_args

    def patched(*a, **k):
        return orig(*a, **k) + ["--enable-new-backend"]

    bu.get_walrus_args = patched


def _build_bass():
    import concourse.bass as bass
    import concourse.mybir as mybir
    import concourse.tile as tile

    _patch_new_backend()

    bf16 = mybir.dt.bfloat16
    f32 = mybir.dt.float32

    nc = bass.Bass()
    xP = nc.declare_dram_parameter("xP", [N, 2 * MPC], bf16, isOutput=False)
    wP = nc.declare_dram_parameter("wP", [N, K], bf16, isOutput=False)
    yT = nc.declare_dram_parameter("yT", [N, MPC], bf16, isOutput=True)

    with tile.TileContext(nc) as tc:
        with (
            tc.tile_pool(name="w", bufs=1) as wpool,
            tc.tile_pool(name="pad", bufs=1) as padpool,
            tc.tile_pool(name="x", bufs=4) as xpool,
            tc.tile_pool(name="y", bufs=2) as ypool,
            tc.tile_pool(name="ps", bufs=4, space=bass.MemorySpace.PSUM) as pspool,
        ):
            # w on the scalar (ACT) HWDGE ring so the sync ring's first
            # descriptor is the first x chunk.
            w_t = wpool.tile([N, K], bf16)
            nc.scalar.dma_start(w_t[:], wP[:, :])

            # Inert: perturbs the BIR so HLO-keyed NEFF caches can't serve
            # a stale binary compiled without the walrus flag above.
            pad_t = padpool.tile([N, 1], f32)
            nc.gpsimd.memset(pad_t[:], 0)

            xcol = 0  # running column offset into xP (2 cols per token)
            for t, chunks in enumerate(TILE_CHUNKS):
                mo = t * MT
                x_t = xpool.tile([N, 2 * MT], bf16)
                # One DMA per chunk; each is one contiguous run per
                # partition on both sides (2*L tokens * 2 B).
                cbase = []  # (tile-token base, SBUF column base, L)
                tcol = 0
                tbase = 0
                for L in chunks:
                    nc.sync.dma_start(
                        x_t[:, tcol : tcol + 2 * L],
                        xP[:, xcol : xcol + 2 * L],
                    )
                    cbase.append((tbase, tcol, L))
                    tbase += L
                    tcol += 2 * L
                    xcol += 2 * L

                y_t = ypool.tile([N, MT], bf16)
                stores = TILE_STORES[t]
                prev_store = 0
                ci = 0

                for i, mb in enumerate(range(0, MT, PB)):
                    while mb >= cbase[ci][0] + cbase[ci][2]:
                        ci += 1
                    cb_tok, cb_col, L = cbase[ci]
                    ps = pspool.tile([N, PB], f32)
                    for o in (0, PT):
                        col = cb_col + (mb - cb_tok) + o
                        nc.tensor.matmul(
                            ps[:, o : o + PT],
                            w_t[:, 0:N],
                            x_t[:, col : col + PT],
                            start=True,
                            stop=False,
                        )
                        nc.tensor.matmul(
                            ps[:, o : o + PT],
                            w_t[:, N : 2 * N],
                            x_t[:, col + L : col + L + PT],
                            start=False,
                            stop=True,
                        )
                    last_block = t == len(TILE_CHUNKS) - 1 and mb == MT - PB
                    if last_block:
                        # Split the final block's cast across both engines
                        # (each half's accumulation group closes on its own)
                        # so the final 512-token store fires as soon as the
                        # scalar half lands — shortest possible tail chain.
                        nc.vector.tensor_copy(
                            y_t[:, mb : mb + PT], ps[:, 0:PT]
                        )
                        nc.scalar.copy(
                            y_t[:, mb + PT : mb + PB], ps[:, PT : 2 * PT]
                        )
                        nc.scalar.dma_start(
                            yT[:, mo + prev_store : mo + mb + PT],
                            y_t[:, prev_store : mb + PT],
                        )
                        nc.scalar.dma_start(
                            yT[:, mo + mb + PT : mo + MT],
                            y_t[:, mb + PT : MT],
                        )
                        continue
                    # Alternate cast engines; odd blocks on scalar so each
                    # store (also on scalar) follows its last cast in
                    # program order on the same engine — no cross-engine
                    # sem hop on the final store.
                    if i % 2 == 1:
                        nc.scalar.copy(y_t[:, mb : mb + PB], ps[:])
                    else:
                        nc.vector.tensor_copy(y_t[:, mb : mb + PB], ps[:])
                    if mb + PB in stores:
                        so, se = prev_store, mb + PB
                        prev_store = se
                        nc.scalar.dma_start(
                            yT[:, mo + so : mo + se], y_t[:, so:se]
                        )

    _split_multi_waits(nc, mybir)
    return nc


_NC_CACHE = None


def _get_nc():
    global _NC_CACHE
    if _NC_CACHE is None:
        _NC_CACHE = _build_bass()
    return _NC_CACHE


def _run(in_maps, **kwargs):
    from concourse.bass_utils import run_bass_kernel_spmd

    return run_bass_kernel_spmd(_get_nc(), in_maps, list(range(G)), **kwargs)


def make_in_maps(x, w):
    x = np.asarray(x)
    w = np.asarray(w)
    in_maps = []
    for g in range(G):
        xg = x[g * MPC : (g + 1) * MPC, :]
        wg = w[g * N : (g + 1) * N, :]
        # Per chunk of L tokens starting at token T:
        #   xP[p, colbase + h*L + t] = xg.T[h*128+p, T+t]
        xgT = xg.T
        segs = []
        T = 0
        for chunks in TILE_CHUNKS:
            for L in chunks:
                seg = xgT[:, T : T + L].reshape(2, N, L)
                segs.append(seg.transpose(1, 0, 2).reshape(N, 2 * L))
                T += L
        xPg = np.ascontiguousarray(np.concatenate(segs, axis=1))
        # wP[p, h*128+n] = wg.T[h*128+p, n]
        wPg = np.ascontiguousarray(
            wg.T.reshape(2, N, N).transpose(1, 0, 2).reshape(N, K)
        )
        in_maps.append({"xP": xPg, "wP": wPg})
    return in_maps


def assemble(results, dtype):
    out = np.zeros((M, G * N), dtype=dtype)
    for g in range(G):
        yTg = np.asarray(results[g]["yT"])
        out[g * MPC : (g + 1) * MPC, g * N : (g + 1) * N] = yTg.T
    return out


def kernel(x, w):
    x = np.asarray(x)
    w = np.asarray(w)
    res = _run(make_in_maps(x, w))
    return assemble(res.results, x.dtype)



# revision 37
# speedup vs baseline: 1.0513x; 1.0513x over previous
"""Grouped GEMM (MoE block-diagonal) on 8 Trainium2 NeuronCores.

Problem: x [262144, 256] bf16, w [1024, 256] bf16 (G=8 experts of [128, 256]).
Rows g*32768:(g+1)*32768 of x belong to expert g.
Output [262144, 1024] bf16, block-diagonal: out[rows_g, g*128:(g+1)*128] = x_g @ w_g^T.

Strategy (expert-parallel):
  - Core g gets expert g: x_g [32768, 256] and w_g [128, 256].
  - Host packs both operands so the contraction dim K lands on SBUF
    partitions (PE matmul contracts over the partition dim) AND every load
    DMA moves ONE contiguous run per partition (per chunk of L tokens at
    token T: xP[p, colbase + h*L + t] = x_g^T[h*128+p, T+t], a 2*L*2-byte
    run).  4 KiB runs cap the HWDGE load stream at ~240 GB/s (~73
    ns/packet fixed cost); the 16 KiB runs of the 4096-token chunks let
    the 16 SDMA engines reach the HBM roofline (~420 GB/s observed).
  - Device computes yT_g [128, 32768] = w_g @ x_g^T with lhsT = w_g^T
    (stationary, both K-halves resident) and rhs = packed x columns,
    512-token matmuls accumulating K over 2 halves into [128,1024] PSUM.
  - Loads and stores taper at the end (final chunks 2048/1024/1024, final
    stores 4096/2048/1024/1024 tokens) so the compute+store tail after the
    load stream ends is short.
  - Host transposes yT_g back and scatters into the zero-filled
    block-diagonal output (the zero blocks never touch the device).
"""

import sys

for _p in ("/opt/trn_rl_repo", "/root/.axon_site/_ro/trn_rl_repo"):
    if _p not in sys.path:
        sys.path.insert(0, _p)

import numpy as np

G = 8          # experts == cores
K = 256        # contraction dim
N = 128        # output dim per expert
M = 262144     # total tokens
MPC = M // G   # tokens per core = 32768

MT = 8192      # tokens per tile
PT = 512       # tokens per matmul (max PE free dim)
PB = 1024      # tokens per PSUM tile

# Load-chunk schedule per tile (token counts, each a PB multiple). The bulk
# uses 4096-token chunks (16 KiB contiguous per partition -> near-line-rate
# descriptors); the final tile tapers so the last chunk's compute+store tail
# after the load stream ends is short.
TILE_CHUNKS = [
    [2048, 2048, 4096],
    [4096, 4096],
    [4096, 4096],
    [4096, 2048, 1024, 1024],
]
# Store boundaries per tile (exclusive token ends within the tile); the last
# tile tapers so the final store is small and lands right after its cast.
TILE_STORES = [
    [8192],
    [8192],
    [8192],
    [2048, 4096, 6144, 7168],
]


def _split_multi_waits(nc, mybir):
    """This walrus build rejects any instruction carrying more than one sync
    wait ("Too many sync wait commands", setupSyncWait). Hoist all but one
    wait of each offender onto fresh single-wait EventSemaphore instructions
    placed just before it on the same engine queue — semantically identical
    (sequencer-level blocking, monotonic sem conditions)."""
    for fn in nc.m.functions:
        for blk in fn.blocks:
            new_insts = []
            for inst in blk.instructions:
                si = getattr(inst, "sync_info", None)
                waits = list(si.on_wait) if si is not None and si.on_wait else []
                if len(waits) > 1:
                    for w in waits[:-1]:
                        name = nc.get_next_instruction_name()
                        ev = mybir.InstEventSemaphore(
                            name=name,
                            engine=inst.engine,
                            ins=[],
                            outs=[],
                            sync_info=mybir.SyncInfo(on_wait=[w], on_update=[]),
                        )
                        nc.inst_map[name] = ev
                        new_insts.append(ev)
                    si.on_wait = waits[-1:]
                new_insts.append(inst)
            blk.instructions = new_insts


def _build_bass():
    import concourse.bass as bass
    import concourse.mybir as mybir
    import concourse.tile as tile

    bf16 = mybir.dt.bfloat16
    f32 = mybir.dt.float32

    nc = bass.Bass()
    xP = nc.declare_dram_parameter("xP", [N, 2 * MPC], bf16, isOutput=False)
    wP = nc.declare_dram_parameter("wP", [N, K], bf16, isOutput=False)
    yT = nc.declare_dram_parameter("yT", [N, MPC], bf16, isOutput=True)

    with tile.TileContext(nc) as tc:
        with (
            tc.tile_pool(name="w", bufs=1) as wpool,
            tc.tile_pool(name="x", bufs=4) as xpool,
            tc.tile_pool(name="y", bufs=2) as ypool,
            tc.tile_pool(name="ps", bufs=4, space=bass.MemorySpace.PSUM) as pspool,
        ):
            # w on the scalar (ACT) HWDGE ring so the sync ring's first
            # descriptor is the first x chunk.
            w_t = wpool.tile([N, K], bf16)
            nc.scalar.dma_start(w_t[:], wP[:, :])

            xcol = 0  # running column offset into xP (2 cols per token)
            # Tile 2's store is deferred until tile 3's first store issues:
            # loads finish sooner without it competing mid-stream, and its
            # already-cast bytes back-fill the drain at full HBM rate while
            # tile 3's casts complete (its y buffer is never recycled, so
            # holding it in SBUF is free).
            deferred = []
            for t, chunks in enumerate(TILE_CHUNKS):
                mo = t * MT
                x_t = xpool.tile([N, 2 * MT], bf16)
                # One DMA per chunk; each is one contiguous run per
                # partition on both sides (2*L tokens * 2 B).
                cbase = []  # (tile-token base, SBUF column base, L)
                tcol = 0
                tbase = 0
                for L in chunks:
                    nc.sync.dma_start(
                        x_t[:, tcol : tcol + 2 * L],
                        xP[:, xcol : xcol + 2 * L],
                    )
                    cbase.append((tbase, tcol, L))
                    tbase += L
                    tcol += 2 * L
                    xcol += 2 * L

                y_t = ypool.tile([N, MT], bf16)
                stores = TILE_STORES[t]
                prev_store = 0
                ci = 0

                for i, mb in enumerate(range(0, MT, PB)):
                    while mb >= cbase[ci][0] + cbase[ci][2]:
                        ci += 1
                    cb_tok, cb_col, L = cbase[ci]
                    ps = pspool.tile([N, PB], f32)
                    for o in (0, PT):
                        col = cb_col + (mb - cb_tok) + o
                        nc.tensor.matmul(
                            ps[:, o : o + PT],
                            w_t[:, 0:N],
                            x_t[:, col : col + PT],
                            start=True,
                            stop=False,
                        )
                        nc.tensor.matmul(
                            ps[:, o : o + PT],
                            w_t[:, N : 2 * N],
                            x_t[:, col + L : col + L + PT],
                            start=False,
                            stop=True,
                        )
                    last_block = t == len(TILE_CHUNKS) - 1 and mb == MT - PB
                    if last_block:
                        # Split the final block's cast across both engines
                        # (each half's accumulation group closes on its own)
                        # so the final 512-token store fires as soon as the
                        # scalar half lands — shortest possible tail chain.
                        nc.vector.tensor_copy(
                            y_t[:, mb : mb + PT], ps[:, 0:PT]
                        )
                        nc.scalar.copy(
                            y_t[:, mb + PT : mb + PB], ps[:, PT : 2 * PT]
                        )
                        nc.scalar.dma_start(
                            yT[:, mo + prev_store : mo + mb + PT],
                            y_t[:, prev_store : mb + PT],
                        )
                        nc.scalar.dma_start(
                            yT[:, mo + mb + PT : mo + MT],
                            y_t[:, mb + PT : MT],
                        )
                        continue
                    # Alternate cast engines; odd blocks on scalar so each
                    # store (also on scalar) follows its last cast in
                    # program order on the same engine — no cross-engine
                    # sem hop on the final store.
                    if i % 2 == 1:
                        nc.scalar.copy(y_t[:, mb : mb + PB], ps[:])
                    else:
                        nc.vector.tensor_copy(y_t[:, mb : mb + PB], ps[:])
                    if mb + PB in stores:
                        so, se = prev_store, mb + PB
                        prev_store = se
                        if t == 2:
                            deferred.append((mo + so, mo + se, y_t, so, se))
                            continue
                        nc.scalar.dma_start(
                            yT[:, mo + so : mo + se], y_t[:, so:se]
                        )
                        for dso, dse, yp, pso, pse in deferred:
                            nc.scalar.dma_start(
                                yT[:, dso:dse], yp[:, pso:pse]
                            )
                        deferred = []

    _split_multi_waits(nc, mybir)
    return nc


_NC_CACHE = None


def _get_nc():
    global _NC_CACHE
    if _NC_CACHE is None:
        _NC_CACHE = _build_bass()
    return _NC_CACHE


def _run(in_maps, **kwargs):
    from concourse.bass_utils import run_bass_kernel_spmd

    return run_bass_kernel_spmd(_get_nc(), in_maps, list(range(G)), **kwargs)


def make_in_maps(x, w):
    x = np.asarray(x)
    w = np.asarray(w)
    in_maps = []
    for g in range(G):
        xg = x[g * MPC : (g + 1) * MPC, :]
        wg = w[g * N : (g + 1) * N, :]
        # Per chunk of L tokens starting at token T:
        #   xP[p, colbase + h*L + t] = xg.T[h*128+p, T+t]
        xgT = xg.T
        segs = []
        T = 0
        for chunks in TILE_CHUNKS:
            for L in chunks:
                seg = xgT[:, T : T + L].reshape(2, N, L)
                segs.append(seg.transpose(1, 0, 2).reshape(N, 2 * L))
                T += L
        xPg = np.ascontiguousarray(np.concatenate(segs, axis=1))
        # wP[p, h*128+n] = wg.T[h*128+p, n]
        wPg = np.ascontiguousarray(
            wg.T.reshape(2, N, N).transpose(1, 0, 2).reshape(N, K)
        )
        in_maps.append({"xP": xPg, "wP": wPg})
    return in_maps


def assemble(results, dtype):
    out = np.zeros((M, G * N), dtype=dtype)
    for g in range(G):
        yTg = np.asarray(results[g]["yT"])
        out[g * MPC : (g + 1) * MPC, g * N : (g + 1) * N] = yTg.T
    return out


def kernel(x, w):
    x = np.asarray(x)
    w = np.asarray(w)
    res = _run(make_in_maps(x, w))
    return assemble(res.results, x.dtype)

